# revision 28
# baseline (speedup 1.0000x reference)
"""Trainium2 Bass kernel for nn_CoreDiffusion (gnn_message_passing).

Sharding: node dim N=4096 split across 8 cores (512 rows each).

Key ideas vs the f32/f16 baseline:
  - adj is quantized on the host to fp8-e3m4 of (adj - 0.5): 1 byte/elem
    halves SBUF-side DMA bytes vs fp16 (DMA cost is SBUF-side bytes).
    The 0.5 shift halves quantization error; the 0.5*colsum(x) correction
    is added back via a k=1 ones-row matmul into the same PSUM group.
  - adj is pre-TRANSPOSED on the host so the PE needs no on-chip
    transpose of adj: matmul lhsT = adjT chunk directly from DRAM.
  - Phase A emits msg as [128 nodes, 64 d] (adjT stationary, x moving):
    out free-size 64/matmul, half the PE rows of the [64, 512] layout.
  - GRU runs on [64, nodes] layout: hx is PE-transposed per 128-node
    block (f16), GRU matmuls use fp16 weights, pointwise in f16.
Per-core budget (cost model): DMA ~51us (floor: 16.8MB adj + 1MB x),
PE ~37us, DVE ~25us, Act ~20us -> DMA-bound.
"""
import numpy as np
import ml_dtypes
from contextlib import ExitStack

import concourse.bass as bass
import concourse.mybir as mybir
import concourse.tile as tile
from concourse import bacc
from concourse.masks import make_identity
from concourse.bass_utils import run_bass_kernel_spmd

F32 = mybir.dt.float32
F16 = mybir.dt.float16
F8 = mybir.dt.float8e3
AF = mybir.ActivationFunctionType
E3M4 = ml_dtypes.float8_e3m4

B, C, N, D, H = 2, 4, 4096, 64, 64
NCORES = 8
NS = N // NCORES            # 512 nodes per core
JC = N // 128               # 32 contraction chunks of 128
NB = NS // 128              # 4 node blocks of 128 per core
LN_EPS = 1e-5


def build():
    nc = bacc.Bacc("TRN2", target_bir_lowering=False, debug=False,
                   num_devices=NCORES)
    adjT8 = nc.declare_dram_parameter("adjT8", [B, C, N, NS], F8, isOutput=False)
    x16d = nc.declare_dram_parameter("x16d", [128, B, JC, D], F16, isOutput=False)
    corr16 = nc.declare_dram_parameter("corr16", [1, B, D], F16, isOutput=False)
    wihT = nc.declare_dram_parameter("wihT", [D, 3 * H], F16, isOutput=False)
    whhT = nc.declare_dram_parameter("whhT", [H, 3 * H], F16, isOutput=False)
    gbias = nc.declare_dram_parameter("gbias", [H, 4], F32, isOutput=False)
    gamma = nc.declare_dram_parameter("gamma", [H], F32, isOutput=False)
    beta = nc.declare_dram_parameter("beta", [H], F32, isOutput=False)
    out32 = nc.declare_dram_parameter("out32", [B, 128, NB, D], F32, isOutput=True)

    with tile.TileContext(nc) as tc, ExitStack() as ctx:
        const = ctx.enter_context(tc.tile_pool(name="const", bufs=1))
        adj_pool = ctx.enter_context(tc.tile_pool(name="adj", bufs=3))
        work = ctx.enter_context(tc.tile_pool(name="work", bufs=2))
        psA = ctx.enter_context(tc.tile_pool(name="psA", bufs=2, space="PSUM"))
        psT = ctx.enter_context(tc.tile_pool(name="psT", bufs=1, space="PSUM"))
        psG = ctx.enter_context(tc.tile_pool(name="psG", bufs=1, space="PSUM"))
        psH = ctx.enter_context(tc.tile_pool(name="psH", bufs=2, space="PSUM"))

        # ---------- constants ----------
        ident = const.tile([128, 128], F32)
        make_identity(nc, ident)
        ident16 = const.tile([128, 128], F16)
        nc.vector.tensor_copy(ident16, ident)
        ones16 = const.tile([1, 128], F16)
        nc.vector.memset(ones16, 1.0)

        x16 = const.tile([128, B, JC, D], F16)
        nc.sync.dma_start(x16, x16d[:, :, :, :])
        corr_sb = const.tile([1, B, D], F16)
        nc.scalar.dma_start(corr_sb, corr16[:, :, :])
        wih_sb = const.tile([D, 3 * H], F16)
        nc.scalar.dma_start(wih_sb, wihT[:, :])
        whh_sb = const.tile([H, 3 * H], F16)
        nc.scalar.dma_start(whh_sb, whhT[:, :])
        gb_sb = const.tile([H, 4], F32)
        nc.scalar.dma_start(gb_sb, gbias[:, :])
        gam_sb = const.tile([128, D], F32)
        g_ap = gamma[:]
        nc.scalar.dma_start(out=gam_sb, in_=bass.AP(
            tensor=g_ap.tensor, offset=g_ap.offset, ap=[[0, 128]] + list(g_ap.ap)))
        bet_sb = const.tile([128, D], F32)
        b_ap = beta[:]
        nc.scalar.dma_start(out=bet_sb, in_=bass.AP(
            tensor=b_ap.tensor, offset=b_ap.offset, ap=[[0, 128]] + list(b_ap.ap)))
        eps_sb = const.tile([128, 1], F32)
        nc.vector.memset(eps_sb, LN_EPS)

        # persistent state
        s_run = const.tile([128, B, NB, D], F32)   # cumsum of msg per b
        h16 = const.tile([H, B, NS], F16)          # GRU hidden
        osum = const.tile([H, B, NS], F32)         # sum over c of h

        def emit_transpose(b, c, hxb):
            """Stage 1 of the pipelined GRU tail: PE transposes of hx plus the
            PSUM->SBUF copy. Slotted between the two DMA-half matmul groups of
            the NEXT chunk so the copy latency hides under its second half."""
            ps_tr = psT.tile([H, NB, 128], F16, tag="tr", name="ps_tr")
            for blk in range(NB):
                nc.tensor.transpose(ps_tr[:, blk, :], hxb[:, blk, :], ident16)
            hx16 = work.tile([H, NS], F16, tag="hx", name="hx16")
            nc.vector.tensor_copy(hx16, ps_tr)
            return hx16

        def emit_gru_rest(b, c, hx16):

            ps_r = psG.tile([H, NS], F32, tag="r", name="ps_r")
            ps_z = psG.tile([H, NS], F32, tag="z", name="ps_z")
            ps_n = psG.tile([H, NS], F32, tag="n", name="ps_n")
            nc.tensor.matmul(ps_r, wih_sb[:, 0:H], hx16,
                             start=True, stop=(c == 0))
            nc.tensor.matmul(ps_z, wih_sb[:, H:2 * H], hx16,
                             start=True, stop=(c == 0))
            nc.tensor.matmul(ps_n, wih_sb[:, 2 * H:3 * H], hx16,
                             start=True, stop=True)
            if c > 0:
                nc.tensor.matmul(ps_r, whh_sb[:, 0:H], h16[:, b],
                                 start=False, stop=True)
                nc.tensor.matmul(ps_z, whh_sb[:, H:2 * H], h16[:, b],
                                 start=False, stop=True)
                ps_hn = psH.tile([H, NS], F32, tag="hn", name="ps_hn")
                nc.tensor.matmul(ps_hn, whh_sb[:, 2 * H:3 * H], h16[:, b],
                                 start=True, stop=True)

            r16 = work.tile([H, NS], F16, tag="r16", name="r16")
            nc.scalar.activation(r16, ps_r, AF.Sigmoid, bias=gb_sb[:, 0:1])
            z16 = work.tile([H, NS], F16, tag="z16", name="z16")
            nc.scalar.activation(z16, ps_z, AF.Sigmoid, bias=gb_sb[:, 1:2])
            n16 = work.tile([H, NS], F16, tag="n16", name="n16")
            if c > 0:
                # t1 = (ps_hn + b_hn) * r   (fused)
                t1 = work.tile([H, NS], F16, tag="t1", name="t1")
                nc.vector.scalar_tensor_tensor(
                    t1, ps_hn, gb_sb[:, 3:4], r16,
                    mybir.AluOpType.add, mybir.AluOpType.mult)
                t2 = work.tile([H, NS], F16, tag="t2", name="t2")
                nc.vector.tensor_add(t2, t1, ps_n)
                nc.scalar.activation(n16, t2, AF.Tanh, bias=gb_sb[:, 2:3])
            else:
                nc.scalar.activation(n16, ps_n, AF.Tanh, bias=gb_sb[:, 2:3])
            # h' = n + z*(h - n);  c==0 (h=0): h' = n - z*n
            if c > 0:
                t3 = work.tile([H, NS], F16, tag="t3", name="t3")
                nc.vector.tensor_sub(t3, h16[:, b], n16)
                t4 = work.tile([H, NS], F16, tag="t4", name="t4")
                nc.vector.tensor_mul(t4, z16, t3)
                nc.vector.tensor_add(h16[:, b], n16, t4)
            else:
                t4 = work.tile([H, NS], F16, tag="t4", name="t4")
                nc.vector.tensor_mul(t4, z16, n16)
                nc.vector.tensor_sub(h16[:, b], n16, t4)
            # last step's osum gates LayerNorm: keep it off the slow Pool path
            eng = nc.vector if c == C - 1 else nc.gpsimd
            if c == 0:
                eng.tensor_copy(osum[:, b], h16[:, b])
            else:
                eng.tensor_add(osum[:, b], osum[:, b], h16[:, b])

        def emit_ln(b):
            # transpose osum (f32) straight into PSUM; stats + normalize
            # read PSUM directly -- no staging copies.
            ps_ln = psT.tile([128, NB, D], F32, tag="tr", name="ps_ln")
            for blk in range(NB):
                nc.tensor.transpose(ps_ln[:, blk, :],
                                    osum[:, b, blk * 128:(blk + 1) * 128],
                                    ident[0:H, 0:H])
            stats = work.tile([128, NB, 6], F32, tag="stats", name="stats")
            mv = work.tile([128, NB, 2], F32, tag="mv", name="mv")
            for blk in range(NB):
                nc.vector.bn_stats(stats[:, blk, :], ps_ln[:, blk, :])
                nc.vector.bn_aggr(mv[:, blk, :], stats[:, blk, :])
            rstd = work.tile([128, NB, 1], F32, tag="rstd", name="rstd")
            nc.scalar.activation(rstd, mv[:, :, 1:2], AF.Sqrt, bias=eps_sb)
            nc.vector.reciprocal(rstd, rstd)
            out_st = work.tile([128, NB, D], F32, tag="out_st", name="out_st")
            for blk in range(NB):
                xm = work.tile([128, D], F32, tag="xm", name="xm")
                # xm = (ps_ln - mu) * gamma ; out = xm * rstd + beta
                nc.vector.scalar_tensor_tensor(
                    xm, ps_ln[:, blk, :], mv[:, blk, 0:1], gam_sb,
                    mybir.AluOpType.subtract, mybir.AluOpType.mult)
                nc.vector.scalar_tensor_tensor(
                    out_st[:, blk, :], xm, rstd[:, blk, :], bet_sb,
                    mybir.AluOpType.mult, mybir.AluOpType.add)
                nc.sync.dma_start(out32[b, :, blk, :], out_st[:, blk, :])

        # Pipeline schedule per iteration k (chunk order c-major, b inner;
        # last c runs b=[1,0] so b=0 is the final chunk):
        #   dma(k); A(k) jc 0..15; transposes(k-1); A(k) jc 16..31;
        #   gru-rest(k-1); cumsum(k)+relu(k)
        # The hx copy of k-1 lands during A(k)'s second half, so the GRU
        # matmuls of k-1 issue with no PE stall; the pointwise chain of k-1
        # overlaps A(k)/A(k+1) on DVE/Act.
        prev_tr = None       # (b, c, hxb) awaiting transpose
        prev_rest = None     # (b, c, hx16) awaiting gru-rest
        for c in range(C):
            for b in ([1, 0] if c == C - 1 else [0, 1]):
                last = c == C - 1 and b == 0
                # ---- DMA adjT chunk (prefetched via pool rotation) ----
                a_t = adj_pool.tile([128, JC, NS], F8, tag="a", name="a_t")
                for h in range(2):
                    nc.sync.dma_start(
                        a_t[:, h * (JC // 2):(h + 1) * (JC // 2), :],
                        adjT8[b, c, h * (N // 2):(h + 1) * (N // 2), :]
                        .rearrange("(q p) i -> p q i", p=128))

                # ---- Phase A: msg[128i, NB, 64d] = adjT.T @ x + 0.5*colsum ----
                ps_m = psA.tile([128, NB, D], F32, tag="m", name="ps_m")
                for blk in range(NB):
                    nc.tensor.matmul(ps_m[:, blk, :], ones16, corr_sb[:, b, :],
                                     start=True, stop=False)
                for jc in range(JC // 2):
                    for blk in range(NB):
                        nc.tensor.matmul(
                            ps_m[:, blk, :],
                            a_t[:, jc, blk * 128:(blk + 1) * 128],
                            x16[:, b, jc, :],
                            start=False, stop=False)
                if prev_tr is not None:
                    prev_rest = (prev_tr[0], prev_tr[1], emit_transpose(*prev_tr))
                    prev_tr = None
                for jc in range(JC // 2, JC):
                    for blk in range(NB):
                        nc.tensor.matmul(
                            ps_m[:, blk, :],
                            a_t[:, jc, blk * 128:(blk + 1) * 128],
                            x16[:, b, jc, :],
                            start=False, stop=(jc == JC - 1))
                # ---- cumsum over c + relu (feeds transpose next iter).
                # Emitted BEFORE the gru-rest of k-1 so cumsum(k) sits ahead
                # of the pointwise chain in the DVE FIFO: the psA buffer WAR
                # for chunk k+2 then clears early.
                if c == 0:
                    nc.vector.tensor_copy(s_run[:, b], ps_m)
                else:
                    nc.vector.tensor_add(s_run[:, b], s_run[:, b], ps_m)
                hxb = work.tile([128, NB, D], F16, tag="hxb", name="hxb")
                if last:
                    nc.vector.tensor_relu(hxb, s_run[:, b])
                else:
                    nc.gpsimd.tensor_relu(hxb, s_run[:, b])
                prev_tr = (b, c, hxb)

                if prev_rest is not None:
                    emit_gru_rest(*prev_rest)
                    prev_rest = None
        # drain: last chunk's transpose + gru + LayerNorms (b=1 first: its
        # chain finished during the final Phase A).
        hx_last = emit_transpose(*prev_tr)
        emit_gru_rest(prev_tr[0], prev_tr[1], hx_last)
        emit_ln(1)
        emit_ln(0)

    nc.compile()
    return nc


_NC_CACHE = None


def _get_nc():
    global _NC_CACHE
    if _NC_CACHE is None:
        _NC_CACHE = build()
    return _NC_CACHE


def _prep_host(inputs):
    """Host-side prep: quantize + transpose adj, cast x, pack weights."""
    adj = np.asarray(inputs["adj"], dtype=np.float32)
    x = np.asarray(inputs["x"], dtype=np.float32)
    # fp8-e3m4 of (adj - 0.5), transposed to [B, C, j, i]
    adjT8_full = np.ascontiguousarray(
        (adj - np.float32(0.5)).transpose(0, 1, 3, 2)).astype(E3M4)
    x16 = x.astype(np.float16)                       # [B, N, D]
    x16d = np.ascontiguousarray(
        x16.reshape(B, JC, 128, D).transpose(2, 0, 1, 3))  # [128, B, JC, D]
    corr = (0.5 * x16.astype(np.float32).sum(axis=1))      # [B, D]
    corr16 = corr.astype(np.float16).reshape(1, B, D)
    w_ih = np.asarray(inputs["w_ih"], dtype=np.float32)
    w_hh = np.asarray(inputs["w_hh"], dtype=np.float32)
    b_ih = np.asarray(inputs["b_ih"], dtype=np.float32)
    b_hh = np.asarray(inputs["b_hh"], dtype=np.float32)
    wihT = np.ascontiguousarray(w_ih.T).astype(np.float16)   # [64, 192]
    whhT = np.ascontiguousarray(w_hh.T).astype(np.float16)
    gb = np.stack([b_ih[0:H] + b_hh[0:H],
                   b_ih[H:2 * H] + b_hh[H:2 * H],
                   b_ih[2 * H:3 * H],
                   b_hh[2 * H:3 * H]], axis=1).astype(np.float32)  # [64, 4]
    common = {
        "x16d": x16d, "corr16": corr16, "wihT": wihT, "whhT": whhT,
        "gbias": gb,
        "gamma": np.asarray(inputs["gamma"], dtype=np.float32),
        "beta": np.asarray(inputs["beta"], dtype=np.float32),
    }
    return adjT8_full, common


def run(inputs, **spmd_kwargs):
    nc = _get_nc()
    adjT8_full, common = _prep_host(inputs)
    in_maps = []
    for k in range(NCORES):
        m = dict(common)
        m["adjT8"] = np.ascontiguousarray(
            adjT8_full[:, :, :, k * NS:(k + 1) * NS])
        in_maps.append(m)
    res = run_bass_kernel_spmd(nc, in_maps, list(range(NCORES)), **spmd_kwargs)
    outs = []
    for k in range(NCORES):
        o = res.results[k]["out32"]                  # [B, 128, NB, D]
        outs.append(o.transpose(0, 2, 1, 3).reshape(B, NS, D))
    out = np.concatenate(outs, axis=1)               # [B, N, H]
    return np.ascontiguousarray(out.astype(np.float32)), res


def kernel(**inputs):
    out, _ = run(inputs)
    return out
